# revision 17
# baseline (speedup 1.0000x reference)
"""Distributed Trainium2 Bass kernel for the CopyDecoder step (8 NeuronCores).

Strategy (tensor parallel, 3 all-gathers):
  - Shard encoder rows (S), attention rows, comb/Ws output rows, LSTM gate
    contributions, Wo output rows, and the copy-score tanh rows across the 8
    cores.  Weights are host-pre-transposed so every matvec runs on the
    TensorEngine with the contraction dim on partitions.
  - AG1 gathers the attention/copy-read partial context vectors (+ softmax
    scalars), AG2 gathers per-core LSTM gate partial sums (so every core gets
    the full gates and computes h1/c1 locally), AG3 gathers per-core softmax
    partition sums for the joint softmax over [V+S] logits.
  - Wo (206 MB total) streams through a rotating SBUF pool and is consumed by
    PE matvecs against h1; everything small hides under that stream.
  - DMA ring split: nc.sync carries the dependency-free weight streams,
    nc.scalar carries big loads consumed after AG1, nc.gpsimd carries the
    dependent flip/bounce chain (so it can stall without blocking streams).
"""
import sys

if '/opt/trn_rl_repo' not in sys.path:
    sys.path.insert(0, '/opt/trn_rl_repo')

import numpy as np

V, E, H, S = 50257, 1024, 1024, 2048
NC = 8
SS = S // NC            # 256 encoder rows per core
HS = H // NC            # 128
VSH = -(-V // NC)       # 6283 logical Wo rows per core
VPAD = 6400             # padded Wo rows per core (12.5 * 512)
NLOG = VPAD + SS        # 6656 logits per core
OUTN = NLOG + H + H     # probs shard + h1 + c1
NEG = -1.0e30

_CACHE = {}


def _build():
    import concourse.bacc as bacc
    import concourse.mybir as mybir
    import concourse.tile as tile

    f32 = mybir.dt.float32
    Alu = mybir.AluOpType
    Act = mybir.ActivationFunctionType

    nc = bacc.Bacc("TRN2", target_bir_lowering=False, debug=False,
                   num_devices=NC)

    def din(name, shape):
        return nc.dram_tensor(name, shape, f32, kind="ExternalInput")

    # ---- external inputs (per-core shards, host-prepared) ----
    x1 = din("x1", [1, E])
    h01 = din("h01", [1, H])
    c01 = din("c01", [1, H])
    xem = din("xem", [1, HS])
    h0m = din("h0m", [1, HS])
    sent = din("sent", [1, SS])
    pcs = din("pcs", [1, SS])
    pw = din("pw", [1, 1])
    attnT = din("attnT", [2 * H, SS])
    attnb = din("attnb", [1, SS])
    enc = din("enc", [SS, 2 * H])
    encT = din("encT", [2 * H, SS])
    combws = din("combws", [2 * H, 2 * HS])
    combb = din("combb", [1, HS])
    wsb = din("wsb", [1, HS])
    wihcat = din("wihcat", [HS, 8, 4, 512])
    bihbhh = din("bihbhh", [2, 4 * H])
    wcT = din("wcT", [2 * H, H])
    wcb = din("wcb", [1, H])
    woT = din("woT", [H, VPAD])
    wob = din("wob", [1, VPAD])
    out = nc.dram_tensor("out", [1, OUTN], f32, kind="ExternalOutput")

    rg = [list(range(NC))]

    with tile.TileContext(nc) as tc:
        with (
            tc.tile_pool(name="dram", bufs=1, space="DRAM") as dram,
            tc.tile_pool(name="vecs", bufs=1) as vp,
            tc.tile_pool(name="rot", bufs=4) as rp,
            tc.tile_pool(name="stm", bufs=2) as st,    # attnT/enc/wc/wo stream
            tc.tile_pool(name="wih", bufs=2) as wh,    # lstm weight stream
            tc.tile_pool(name="psS", bufs=3, space="PSUM") as psS,
            tc.tile_pool(name="psT", bufs=3, space="PSUM") as psT,
        ):
            # ---------------- DRAM scratch / collective bounce ----------
            er_d = dram.tile([1, 512], f32, tag="er_d")
            cs_d = dram.tile([1, 4096], f32, tag="cs_d")
            asv_d = dram.tile([1, 256], f32, tag="asv_d")
            h1_d = dram.tile([1, H], f32, tag="h1_d")
            b1in = dram.tile([1, 4112], f32, tag="b1in")
            g1 = dram.tile([NC, 4112], f32, tag="g1")
            b2in = dram.tile([1, 4 * H], f32, tag="b2in")
            g2 = dram.tile([NC, 4 * H], f32, tag="g2")
            b3in = dram.tile([1, 8], f32, tag="b3in")
            g3 = dram.tile([NC, 8], f32, tag="g3")

            # ---------------- small vector loads (sync ring) ------------
            xh = vp.tile([128, 16], f32, tag="xh")
            nc.sync.dma_start(xh[:, 0:8], x1[:].rearrange("a (c p) -> p (a c)", p=128))
            nc.sync.dma_start(xh[:, 8:16], h01[:].rearrange("a (c p) -> p (a c)", p=128))
            xhm = vp.tile([128, 2], f32, tag="xhm")
            nc.sync.dma_start(xhm[:, 0:1], xem[:].rearrange("a (c p) -> p (a c)", p=128))
            nc.sync.dma_start(xhm[:, 1:2], h0m[:].rearrange("a (c p) -> p (a c)", p=128))
            sent_t = vp.tile([1, SS], f32, tag="sent_t")
            nc.sync.dma_start(sent_t[:], sent[:])
            pcs_t = vp.tile([1, SS], f32, tag="pcs_t")
            nc.sync.dma_start(pcs_t[:], pcs[:])
            pw_t = vp.tile([1, 1], f32, tag="pw_t")
            nc.sync.dma_start(pw_t[:], pw[:])
            attnb_t = vp.tile([1, SS], f32, tag="attnb_t")
            nc.sync.dma_start(attnb_t[:], attnb[:])
            combb_t = vp.tile([1, HS], f32, tag="combb_t")
            nc.sync.dma_start(combb_t[:], combb[:])
            wsb_t = vp.tile([1, HS], f32, tag="wsb_t")
            nc.sync.dma_start(wsb_t[:], wsb[:])
            wcb_t = vp.tile([128, 8], f32, tag="wcb_t")
            nc.sync.dma_start(wcb_t[:], wcb[:].rearrange("a (c p) -> p (a c)", p=128))
            c01_t = vp.tile([1, H], f32, tag="c01_t")
            nc.sync.dma_start(c01_t[:], c01[:])
            ones1 = vp.tile([1, 1], f32, tag="ones1")
            nc.vector.memset(ones1[:], 1.0)
            ones2 = vp.tile([2, 1], f32, tag="ones2")
            nc.vector.memset(ones2[:], 1.0)
            ones8 = vp.tile([NC, 1], f32, tag="ones8")
            nc.vector.memset(ones8[:], 1.0)

            # ------------- big dep-free loads --------------------------
            # sync ring: attnT, enc then (later) wc, wo through the stream pool
            attnT_t = st.tile([128, 16, SS], f32, tag="st")
            nc.sync.dma_start(attnT_t[:], attnT[:].rearrange("(t p) s -> p t s", p=128))
            enc_t = st.tile([128, 2, 2 * H], f32, tag="st")
            nc.sync.dma_start(enc_t[:], enc[:].rearrange("(t p) n -> p t n", p=128))
            # scalar ring: combws, encT, wih chunks
            combws_t = vp.tile([128, 16, 2 * HS], f32, tag="combws_t")
            nc.scalar.dma_start(combws_t[:], combws[:].rearrange("(t p) j -> p t j", p=128))
            encT_t = vp.tile([128, 16, SS], f32, tag="encT_t")
            nc.scalar.dma_start(encT_t[:], encT[:].rearrange("(t p) s -> p t s", p=128))
            wih_ch = []
            for n in range(8):
                whc = wh.tile([128, 4, 512], f32, tag="wh", name=f"whc{n}")
                nc.scalar.dma_start(whc[:], wihcat[:, n, :, :])
                wih_ch.append(whc)

            # ---------------- P1: attention + copy-read partials --------
            att_ps = psS.tile([1, 512], f32, tag="psS", name="att_ps")
            for t in range(16):
                nc.tensor.matmul(att_ps[0:1, 0:SS], xh[:, t:t + 1], attnT_t[:, t, :],
                                 start=(t == 0), stop=False)
            nc.tensor.matmul(att_ps[0:1, 0:SS], ones1[:], attnb_t[:],
                             start=False, stop=True)
            scal16 = vp.tile([1, 16], f32, tag="scal16")
            nc.vector.memset(scal16[:], 0.0)
            sume = scal16[0:1, 0:1]
            e_t = vp.tile([1, SS], f32, tag="e_t")
            nc.scalar.activation(e_t[:], att_ps[0:1, 0:SS], Act.Exp,
                                 accum_out=sume)
            npw = vp.tile([1, 1], f32, tag="npw")
            nc.scalar.mul(npw[:], pw_t[0:1, 0:1], -1.0)
            dabs = vp.tile([1, SS], f32, tag="dabs")
            nc.scalar.activation(dabs[:], sent_t[:], Act.Abs, bias=npw[0:1, 0:1])
            mask_t = vp.tile([1, SS], f32, tag="mask_t")
            nc.scalar.activation(mask_t[:], dabs[:], Act.Relu,
                                 bias=ones1[0:1, 0:1], scale=-1.0)
            rho_t = vp.tile([1, SS], f32, tag="rho_t")
            nc.vector.tensor_tensor(rho_t[:], mask_t[:], pcs_t[:], Alu.mult)
            nc.gpsimd.dma_start(er_d[0:1, 0:SS], e_t[:])
            nc.gpsimd.dma_start(er_d[0:1, SS:2 * SS], rho_t[:])
            er_sb = vp.tile([128, 4], f32, tag="er_sb")
            nc.gpsimd.dma_start(er_sb[:], er_d[:].rearrange("a (c p) -> p (a c)", p=128))
            ones128 = vp.tile([128, 1], f32, tag="ones128")
            nc.vector.memset(ones128[:], 1.0)
            rs_ps = psS.tile([1, 512], f32, tag="psS", name="rs_ps")
            nc.tensor.matmul(rs_ps[0:1, 0:4], ones128[:], er_sb[:],
                             start=True, stop=True)
            nc.scalar.activation(scal16[0:1, 1:3], rs_ps[0:1, 2:4], Act.Copy)
            for n in range(4):
                pp2 = psS.tile([2, 512], f32, tag="psS", name=f"pp2_{n}")
                for t in range(2):
                    nc.tensor.matmul(pp2[:], er_sb[:, t:t + 3:2],
                                     enc_t[:, t, n * 512:(n + 1) * 512],
                                     start=(t == 0), stop=(t == 1))
                pps = rp.tile([2, 512], f32, tag="s512", name=f"pps{n}")
                nc.scalar.activation(pps[:], pp2[:], Act.Copy)
                nc.gpsimd.dma_start(b1in[0:1, n * 512:(n + 1) * 512], pps[0:1, :])
                nc.gpsimd.dma_start(b1in[0:1, 2048 + n * 512:2048 + (n + 1) * 512],
                                    pps[1:2, :])
            nc.gpsimd.dma_start(b1in[0:1, 4096:4112], scal16[:])
            nc.gpsimd.collective_compute(
                "AllGather", Alu.bypass, replica_groups=rg,
                ins=[b1in[:].opt()], outs=[g1[:].opt()])

            # ---------------- P2: combine partials; attentive/selective -
            g1t = vp.tile([NC, 4112], f32, tag="gband", name="g1t")
            nc.gpsimd.dma_start(g1t[:], g1[:])
            scz_ps = psS.tile([1, 512], f32, tag="psS", name="scz_ps")
            nc.tensor.matmul(scz_ps[0:1, 0:16], ones8[:], g1t[:, 4096:4112],
                             start=True, stop=True)
            zatt = vp.tile([1, 2], f32, tag="zatt")
            nc.scalar.activation(zatt[0:1, 0:1], scz_ps[0:1, 0:1], Act.Copy)
            srv = vp.tile([1, 2], f32, tag="srv")
            nc.scalar.activation(srv[:], scz_ps[0:1, 1:3], Act.Copy)
            nc.vector.tensor_tensor(zatt[0:1, 1:2], srv[0:1, 0:1],
                                    srv[0:1, 1:2], Alu.add)
            rz_att = vp.tile([1, 1], f32, tag="rz_att")
            nc.vector.reciprocal(rz_att[:], zatt[0:1, 0:1])
            eps_t = vp.tile([1, 1], f32, tag="eps_t")
            nc.vector.memset(eps_t[:], 1e-9)
            rr_t = vp.tile([1, 1], f32, tag="rr_t")
            nc.vector.tensor_tensor(rr_t[:], zatt[0:1, 1:2], eps_t[:], Alu.add)
            rrho = vp.tile([1, 1], f32, tag="rrho")
            nc.vector.reciprocal(rrho[:], rr_t[:])
            for n in range(8):
                sc = rz_att if n < 4 else rrho
                cps = psS.tile([1, 512], f32, tag="psS", name=f"cps{n}")
                nc.tensor.matmul(cps[:], ones8[:], g1t[:, n * 512:(n + 1) * 512],
                                 start=True, stop=True)
                csl = rp.tile([1, 512], f32, tag="s512", name=f"csl{n}")
                nc.scalar.mul(csl[:], cps[:], sc[0:1, 0:1])
                nc.gpsimd.dma_start(cs_d[0:1, n * 512:(n + 1) * 512], csl[:])
            cs_sb = vp.tile([128, 32], f32, tag="cs_sb")
            nc.gpsimd.dma_start(cs_sb[:], cs_d[:].rearrange("a (c p) -> p (a c)", p=128))
            av_ps = psS.tile([1, 512], f32, tag="psS", name="av_ps")
            for t in range(16):
                nc.tensor.matmul(av_ps[0:1, 0:HS], cs_sb[:, t:t + 1],
                                 combws_t[:, t, 0:HS], start=(t == 0), stop=(t == 15))
            sv_ps = psS.tile([1, 512], f32, tag="psS", name="sv_ps")
            for t in range(16):
                nc.tensor.matmul(sv_ps[0:1, 0:HS], cs_sb[:, 16 + t:17 + t],
                                 combws_t[:, t, HS:2 * HS], start=(t == 0), stop=(t == 15))
            av_sb = vp.tile([1, HS], f32, tag="av_sb")
            nc.vector.tensor_tensor(av_sb[:], av_ps[0:1, 0:HS], combb_t[:], Alu.add)
            sv_sb = vp.tile([1, HS], f32, tag="sv_sb")
            nc.vector.tensor_tensor(sv_sb[:], sv_ps[0:1, 0:HS], wsb_t[:], Alu.add)
            nc.gpsimd.dma_start(asv_d[0:1, 0:HS], av_sb[:])
            nc.gpsimd.dma_start(asv_d[0:1, HS:2 * HS], sv_sb[:])
            asv_sb = vp.tile([128, 2], f32, tag="asv_sb")
            nc.gpsimd.dma_start(asv_sb[:], asv_d[:].rearrange("a (c p) -> p (a c)", p=128))

            # ---------------- P3: LSTM gate partial sums ----------------
            for n in range(8):
                gp_ps = psS.tile([1, 512], f32, tag="psS", name=f"gp_ps{n}")
                nc.tensor.matmul(gp_ps[:], xhm[:, 0:1], wih_ch[n][:, 0, :],
                                 start=True, stop=False)
                nc.tensor.matmul(gp_ps[:], asv_sb[:, 1:2], wih_ch[n][:, 1, :],
                                 start=False, stop=False)
                nc.tensor.matmul(gp_ps[:], asv_sb[:, 0:1], wih_ch[n][:, 2, :],
                                 start=False, stop=False)
                nc.tensor.matmul(gp_ps[:], xhm[:, 1:2], wih_ch[n][:, 3, :],
                                 start=False, stop=True)
                gps = rp.tile([1, 512], f32, tag="s512", name=f"gps{n}")
                nc.scalar.activation(gps[:], gp_ps[:], Act.Copy)
                nc.gpsimd.dma_start(b2in[0:1, n * 512:(n + 1) * 512], gps[:])
            nc.gpsimd.collective_compute(
                "AllGather", Alu.bypass, replica_groups=rg,
                ins=[b2in[:].opt()], outs=[g2[:].opt()])

            # ---------------- P4: full gates -> h1, c1 ------------------
            g2t = vp.tile([NC, 4 * H], f32, tag="gband", name="g2t")
            nc.gpsimd.dma_start(g2t[:], g2[:])
            gact = [None] * 4
            names = ["si", "sf", "tg", "so"]
            funcs = [Act.Sigmoid, Act.Sigmoid, Act.Tanh, Act.Sigmoid]
            for i in range(4):
                gact[i] = vp.tile([1, H], f32, tag="g4", bufs=4, name=names[i])
            for n in range(8):
                bbsl = rp.tile([2, 512], f32, tag="s512", name=f"bbsl{n}")
                nc.gpsimd.dma_start(bbsl[:], bihbhh[:, n * 512:(n + 1) * 512])
                gt_ps = psS.tile([1, 512], f32, tag="psS", name=f"gt_ps{n}")
                nc.tensor.matmul(gt_ps[:], ones8[:], g2t[:, n * 512:(n + 1) * 512],
                                 start=True, stop=False)
                nc.tensor.matmul(gt_ps[:], ones2[:], bbsl[:],
                                 start=False, stop=True)
                g_i = n // 2
                off = (n % 2) * 512
                nc.scalar.activation(gact[g_i][0:1, off:off + 512], gt_ps[:],
                                     funcs[g_i])
            si, sf, tg, so = gact
            c1a = vp.tile([1, H], f32, tag="t3", bufs=2, name="c1a")
            nc.vector.tensor_tensor(c1a[:], sf[:], c01_t[:], Alu.mult)
            c1b = vp.tile([1, H], f32, tag="t3", bufs=2, name="c1b")
            nc.vector.tensor_tensor(c1b[:], si[:], tg[:], Alu.mult)
            c1v = vp.tile([1, H], f32, tag="c1v")
            nc.vector.tensor_tensor(c1v[:], c1a[:], c1b[:], Alu.add)
            th = vp.tile([1, H], f32, tag="t3", bufs=2, name="th")
            nc.scalar.activation(th[:], c1v[:], Act.Tanh)
            h1v = vp.tile([1, H], f32, tag="h1v")
            nc.vector.tensor_tensor(h1v[:], so[:], th[:], Alu.mult)
            nc.gpsimd.dma_start(h1_d[:], h1v[:])
            h1p = vp.tile([128, 8], f32, tag="h1p")
            nc.gpsimd.dma_start(h1p[:], h1_d[:].rearrange("a (c p) -> p (a c)", p=128))

            # ---------------- P5a: copy-score tanh matrix (h1-free) -----
            wc_view = wcT[:].rearrange("(t p) j -> p t j", p=128)
            ttA = vp.tile([128, 8, SS], f32, tag="ttA")
            ttB = vp.tile([128, 8, SS], f32, tag="ttB")
            pingpong = [(None, ttB), (ttB, ttA), (ttA, ttB), (ttB, ttA)]
            for q in range(4):
                wcc = st.tile([128, 4, H], f32, tag="st", name=f"wcch{q}")
                nc.sync.dma_start(wcc[:], wc_view[:, 4 * q:4 * q + 4, :])
                src, dst = pingpong[q]
                for jt in range(8):
                    tq = psT.tile([128, SS], f32, tag="psT", name=f"ttps{q}_{jt}")
                    for i in range(4):
                        nc.tensor.matmul(tq[:],
                                         wcc[:, i, jt * 128:(jt + 1) * 128],
                                         encT_t[:, 4 * q + i, :],
                                         start=(i == 0), stop=(i == 3))
                    if q == 0:
                        nc.scalar.activation(dst[:, jt, :], tq[:], Act.Copy)
                    else:
                        nc.vector.tensor_tensor(dst[:, jt, :], src[:, jt, :],
                                                tq[:], Alu.add)
            tt_sb = ttB
            for jt in range(8):
                nc.scalar.activation(tt_sb[:, jt, :], ttA[:, jt, :],
                                     Act.Tanh, bias=wcb_t[:, jt:jt + 1])

            # ---------------- P5b: logits, exp, local partition sum -----
            lx_sb = vp.tile([1, NLOG], f32, tag="lx_sb")
            zp_sb = vp.tile([1, 16], f32, tag="zp_sb")
            sc_ps = psS.tile([1, 512], f32, tag="psS", name="sc_ps")
            for jt in range(8):
                nc.tensor.matmul(sc_ps[0:1, 0:SS], h1p[:, jt:jt + 1],
                                 tt_sb[:, jt, :], start=(jt == 0), stop=(jt == 7))
            nc.scalar.activation(lx_sb[0:1, VPAD:NLOG], sc_ps[0:1, 0:SS],
                                 Act.Exp, accum_out=zp_sb[0:1, 13:14])
            wo_view = woT[:].rearrange("(t p) v -> p t v", p=128)
            for ch in range(13):
                n0 = ch * 512
                nn = 512 if ch < 12 else 256
                wo_ch = st.tile([128, 8, 512], f32, tag="st", name=f"woch{ch}")
                nc.sync.dma_start(wo_ch[:, :, 0:nn], wo_view[:, :, n0:n0 + nn])
                wobs = rp.tile([1, 512], f32, tag="wob", bufs=2, name=f"wobs{ch}")
                nc.sync.dma_start(wobs[0:1, 0:nn], wob[0:1, n0:n0 + nn])
                wo_ps = psS.tile([1, 512], f32, tag="psS", name=f"wo_ps{ch}")
                for t in range(8):
                    nc.tensor.matmul(wo_ps[0:1, 0:nn], h1p[:, t:t + 1],
                                     wo_ch[:, t, 0:nn], start=(t == 0), stop=False)
                nc.tensor.matmul(wo_ps[0:1, 0:nn], ones1[:], wobs[0:1, 0:nn],
                                 start=False, stop=True)
                nc.scalar.activation(lx_sb[0:1, n0:n0 + nn], wo_ps[0:1, 0:nn],
                                     Act.Exp, accum_out=zp_sb[0:1, ch:ch + 1])
            zl_sb = vp.tile([1, 8], f32, tag="zl_sb")
            nc.vector.memset(zl_sb[:], 0.0)
            zscr = vp.tile([1, 16], f32, tag="zscr")
            nc.scalar.activation(zscr[0:1, 0:14], zp_sb[0:1, 0:14], Act.Abs,
                                 accum_out=zl_sb[0:1, 0:1])
            nc.gpsimd.dma_start(b3in[:], zl_sb[:])
            nc.gpsimd.collective_compute(
                "AllGather", Alu.bypass, replica_groups=rg,
                ins=[b3in[:].opt()], outs=[g3[:].opt()])

            # ---------------- P6: global softmax scale + outputs --------
            z8 = vp.tile([NC, 1], f32, tag="z8")
            nc.gpsimd.dma_start(z8[:], g3[:, 0:1])
            zt_ps = psS.tile([1, 512], f32, tag="psS", name="zt_ps")
            nc.tensor.matmul(zt_ps[0:1, 0:1], z8[:], ones8[:], start=True, stop=True)
            zt_sb = vp.tile([1, 1], f32, tag="zt_sb")
            nc.scalar.activation(zt_sb[:], zt_ps[0:1, 0:1], Act.Copy)
            rz = vp.tile([1, 1], f32, tag="rz")
            nc.vector.reciprocal(rz[:], zt_sb[:])
            for ch in range(13):
                n0 = ch * 512
                prs = rp.tile([1, 512], f32, tag="s512", name=f"prs{ch}")
                nc.scalar.mul(prs[:], lx_sb[0:1, n0:n0 + 512], rz[0:1, 0:1])
                nc.gpsimd.dma_start(out[0:1, n0:n0 + 512], prs[:])
            nc.gpsimd.dma_start(out[0:1, NLOG:NLOG + H], h1v[:])
            nc.gpsimd.dma_start(out[0:1, NLOG + H:OUTN], c1v[:])

    nc.compile()
    return nc


def _get_nc():
    if "nc" not in _CACHE:
        _CACHE["nc"] = _build()
    return _CACHE["nc"]


def _prep_inputs(x, encoder_outputs, sentence, prev_probs, h0, c0,
                 attn_W, attn_b, comb_W, comb_b, Ws_W, Ws_b,
                 Wo_W, Wo_b, Wc_W, Wc_b, W_ih, W_hh, b_ih, b_hh, prev_word):
    f = np.float32
    ca = np.ascontiguousarray

    x = np.asarray(x, f).reshape(1, E)
    h0 = np.asarray(h0, f).reshape(1, H)
    c0 = np.asarray(c0, f).reshape(1, H)
    enc = np.asarray(encoder_outputs, f)
    sent_f = np.asarray(sentence).astype(f)
    probs_c = np.asarray(prev_probs, f)[V:]
    pw_f = np.array([[float(prev_word)]], f)
    attn_W = np.asarray(attn_W, f)
    attn_b = np.asarray(attn_b, f)
    comb_W = np.asarray(comb_W, f)
    comb_b = np.asarray(comb_b, f)
    Ws_W = np.asarray(Ws_W, f)
    Ws_b = np.asarray(Ws_b, f)
    Wo_W = np.asarray(Wo_W, f)
    Wo_b = np.asarray(Wo_b, f)
    Wc_W = np.asarray(Wc_W, f)
    Wc_b = np.asarray(Wc_b, f)
    W_ih = np.asarray(W_ih, f)
    W_hh = np.asarray(W_hh, f)
    b_ih = np.asarray(b_ih, f).reshape(4 * H)
    b_hh = np.asarray(b_hh, f).reshape(4 * H)

    wcT = ca(Wc_W.T)                      # [2H, H], replicated
    wcb = Wc_b.reshape(1, H)
    wihT = W_ih.T                         # [E+2H, 4H]
    whhT = W_hh.T                         # [H, 4H]
    bihbhh = ca(np.stack([b_ih, b_hh]))   # [2, 4H]

    in_maps = []
    for c in range(NC):
        s0 = c * SS
        r0 = c * HS
        v0 = c * VSH
        nv = min(VSH, V - v0)
        woT_s = np.zeros((H, VPAD), f)
        woT_s[:, :nv] = Wo_W[v0:v0 + nv, :].T
        wob_s = np.full((1, VPAD), NEG, f)
        wob_s[0, :nv] = Wo_b[v0:v0 + nv]
        wihcat = ca(np.stack([
            wihT[r0:r0 + HS, :].reshape(HS, 8, 512),
            wihT[E + r0:E + r0 + HS, :].reshape(HS, 8, 512),
            wihT[E + H + r0:E + H + r0 + HS, :].reshape(HS, 8, 512),
            whhT[r0:r0 + HS, :].reshape(HS, 8, 512),
        ], axis=2))                        # [HS, 8, 4, 512]
        combws = ca(np.concatenate(
            [comb_W[r0:r0 + HS, :].T, Ws_W[r0:r0 + HS, :].T], axis=1))
        m = {
            "x1": x, "h01": h0, "c01": c0,
            "xem": ca(x[:, r0:r0 + HS]), "h0m": ca(h0[:, r0:r0 + HS]),
            "sent": ca(sent_f[s0:s0 + SS].reshape(1, SS)),
            "pcs": ca(probs_c[s0:s0 + SS].reshape(1, SS)),
            "pw": pw_f,
            "attnT": ca(attn_W[s0:s0 + SS, :].T),
            "attnb": ca(attn_b[s0:s0 + SS].reshape(1, SS)),
            "enc": ca(enc[s0:s0 + SS, :]),
            "encT": ca(enc[s0:s0 + SS, :].T),
            "combws": combws,
            "combb": ca(comb_b[r0:r0 + HS].reshape(1, HS)),
            "wsb": ca(Ws_b[r0:r0 + HS].reshape(1, HS)),
            "wihcat": wihcat,
            "bihbhh": bihbhh,
            "wcT": wcT, "wcb": wcb,
            "woT": woT_s, "wob": wob_s,
        }
        in_maps.append(m)
    return in_maps


def _unshard(results):
    probs_g = []
    probs_s = []
    for c in range(NC):
        o = np.asarray(results[c]["out"]).reshape(OUTN)
        v0 = c * VSH
        nv = min(VSH, V - v0)
        probs_g.append(o[:nv])
        probs_s.append(o[VPAD:VPAD + SS])
    o0 = np.asarray(results[0]["out"]).reshape(OUTN)
    probs = np.concatenate(probs_g + probs_s).astype(np.float32)
    h1 = o0[NLOG:NLOG + H].reshape(1, H).astype(np.float32)
    c1 = o0[NLOG + H:OUTN].reshape(1, H).astype(np.float32)
    return probs, h1, c1


def run(trace=False, **inputs):
    from concourse import bass_utils
    nc = _get_nc()
    in_maps = _prep_inputs(**inputs)
    res = bass_utils.run_bass_kernel_spmd(
        nc, in_maps, core_ids=list(range(NC)), trace=trace)
    return _unshard(res.results), res


def kernel(**inputs):
    (probs, h1, c1), _ = run(trace=False, **inputs)
    return probs, h1, c1
